# Initial kernel scaffold
#
"""Multi-head attention (B=1, S=4096, D=768, H=12, Hd=64) on 8 trn2 cores.

Sharding: 2 head-groups (6 heads = 384 dims, Megatron column-split wq/wk/wv,
row-split ww) x 4 query-chunks (1024 rows).  core = g*4 + c.
Each core returns a partial output [1024, 768]; host sums the 2 group
partials per chunk and adds (bv @ ww.T + bw).

Device layout (per core):
  xT  [768, 4096]   x transposed (keys/values source)
  xqT [768, 1024]   this core's query columns of xT
  QT/KT [128, 3, S] head-pair-packed transposed projections
                    (partition l, pair p) = local dim p*128+l; pairs of heads
                    share the PE array via row/col tile_position packing.
  V   [128, 32, 384] value rows (key j on partitions, [jtile], local dim)
  scoresT psum [128 keys, q] = K_h Q_h^T; exp on ACT (scale=1/8 folded,
                    no max subtraction: |scores| < 3); attnV accumulates
                    out^T [dims, q] over key tiles in PSUM.
  softmax denominators: DVE-accumulate exp tiles over key tiles, then an
                    all-ones [128,128] matmul broadcast-sums across
                    partitions; normalize after attnV, before out-proj.
"""

import sys

if "/opt/trn_rl_repo" not in sys.path:
    sys.path.insert(0, "/opt/trn_rl_repo")

import numpy as np

import concourse.bass as bass
import concourse.mybir as mybir
import concourse.tile as tile
from concourse.bass_utils import run_bass_kernel_spmd
from concourse.vector_clock import ScopedClock

F32 = mybir.dt.float32
F32R = mybir.dt.float32r  # full-rate fp32 matmul mode (moving dim >= 256)

S = 4096          # sequence length
D = 768           # model dim
NG = 2            # head groups (cores axis 1)
NC = 4            # query chunks (cores axis 2)
DH = D // NG      # dims per group = 384
NP = DH // 128    # head pairs per group = 3
SQ = S // NC      # queries per core = 1024
KO = D // 128     # contraction subtiles = 6
NJ = S // 128     # key tiles = 32
AF = mybir.ActivationFunctionType
SCALE = 0.125     # 1/sqrt(64)

_PATCHED = False


def _patch_drain():
    """walrus in this container rejects >1 sync-wait per instruction
    ("Too many sync wait commands").  TileContext's tail drain aggregates one
    wait per live tile semaphore; redistribute them one-per-nop."""
    global _PATCHED
    if _PATCHED:
        return
    _PATCHED = True

    def _drain_and_barrier(self, tick_clock, wait_clock):
        nc = self.nc
        drain_inst = nc.sync.drain()
        wait_clock.add_sem_waits(
            drain_inst.ins, ScopedClock({None: tick_clock.global_clock})
        )
        si = drain_inst.ins.sync_info
        waits = list(si.on_wait) if si is not None else []
        if len(waits) > 1:
            drain_inst.ins.sync_info = mybir.SyncInfo(
                on_wait=[waits[0]], on_update=list(si.on_update)
            )
            for w in waits[1:]:
                nop = nc.sync.nop(nofuse=True)
                nop.ins.sync_info = mybir.SyncInfo(on_wait=[w], on_update=[])
        nc.all_engine_barrier()
        assert self.sems is not None
        popped = nc._tile_sem_poison_stack.pop()
        assert popped is self._sem_poison
        nc.clear_and_free_semaphores(list(self.sems.allocated().values()))
        nc.all_engine_barrier()

    tile.TileContext._drain_and_barrier = _drain_and_barrier


def _r(ap):
    return ap.bitcast(F32R)


def build_nc():
    _patch_drain()
    nc = bass.Bass("TRN2", target_bir_lowering=False)

    xT = nc.dram_tensor("xT", [D, S], F32, kind="ExternalInput")
    xqT = nc.dram_tensor("xqT", [D, SQ], F32, kind="ExternalInput")
    wqT = nc.dram_tensor("wqT", [D, DH], F32, kind="ExternalInput")
    wkT = nc.dram_tensor("wkT", [D, DH], F32, kind="ExternalInput")
    wvT = nc.dram_tensor("wvT", [D, DH], F32, kind="ExternalInput")
    wwT = nc.dram_tensor("wwT", [DH, D], F32, kind="ExternalInput")
    bq = nc.dram_tensor("bq", [128, NP], F32, kind="ExternalInput")
    bk = nc.dram_tensor("bk", [128, NP], F32, kind="ExternalInput")
    out = nc.dram_tensor("out", [SQ, D], F32, kind="ExternalOutput")

    xT_r = xT.rearrange("(ko p) n -> p ko n", p=128)
    xqT_r = xqT.rearrange("(ko p) n -> p ko n", p=128)
    wqT_r = wqT.rearrange("(ko p) m -> p ko m", p=128)
    wkT_r = wkT.rearrange("(ko p) m -> p ko m", p=128)
    wvT_r = wvT.rearrange("(ko p) m -> p ko m", p=128)
    wwT_r = wwT.rearrange("(p l) o -> l p o", l=128)

    with tile.TileContext(nc) as tc:
        import contextlib

        ctx = contextlib.ExitStack()
        with ctx:
            persist = ctx.enter_context(tc.tile_pool(name="persist", bufs=1))
            # persistent SBUF tensors
            KT = persist.tile([128, NP, S], F32)       # 48KB/part
            V = persist.tile([128, NJ, DH], F32)       # 48KB/part
            QT = persist.tile([128, NP, SQ], F32)      # 12KB/part
            yT = persist.tile([128, NP, SQ], F32)      # 12KB/part
            wk_sb = persist.tile([128, KO, DH], F32)
            wv_sb = persist.tile([128, KO, DH], F32)
            ww_sb = persist.tile([128, NP, D], F32)
            bq_sb = persist.tile([128, NP], F32)
            bk_sb = persist.tile([128, NP], F32)
            ones_sb = persist.tile([128, 128], F32)

            nc.sync.dma_start(wk_sb[:], wkT_r[:])
            nc.sync.dma_start(wv_sb[:], wvT_r[:])
            nc.sync.dma_start(ww_sb[:], wwT_r[:])
            nc.sync.dma_start(bq_sb[:], bq[:])
            nc.sync.dma_start(bk_sb[:], bk[:])
            nc.vector.memset(ones_sb[:], 1.0)

            # ---------------- phase 1: Q projection -> QT ----------------
            with tc.tile_pool(name="ph1", bufs=1) as ph1, \
                 tc.tile_pool(name="ps12", bufs=3, space="PSUM") as ps12:
                wq_sb = ph1.tile([128, KO, DH], F32)
                xq_sb = ph1.tile([128, KO, SQ], F32)
                nc.sync.dma_start(wq_sb[:], wqT_r[:])
                nc.sync.dma_start(xq_sb[:], xqT_r[:])
                for p in range(NP):
                    for n in range(SQ // 512):
                        ps = ps12.tile([128, 512], F32, tag="qk")
                        for ko in range(KO):
                            nc.tensor.matmul(
                                ps[:],
                                _r(wq_sb[:, ko, p * 128:(p + 1) * 128]),
                                _r(xq_sb[:, ko, n * 512:(n + 1) * 512]),
                                start=(ko == 0), stop=(ko == KO - 1),
                            )
                        nc.vector.tensor_scalar_add(
                            QT[:, p, n * 512:(n + 1) * 512], ps[:],
                            bq_sb[:, p:p + 1],
                        )

                # ------------- phase 2: K/V projections (stream xT) -------
                with tc.tile_pool(name="xstream", bufs=3) as xs_pool:
                    for n in range(S // 512):
                        xb = xs_pool.tile([128, KO, 512], F32, tag="xb")
                        nc.sync.dma_start(xb[:], xT_r[:, :, n * 512:(n + 1) * 512])
                        for p in range(NP):
                            ps = ps12.tile([128, 512], F32, tag="qk")
                            for ko in range(KO):
                                nc.tensor.matmul(
                                    ps[:],
                                    _r(wk_sb[:, ko, p * 128:(p + 1) * 128]),
                                    _r(xb[:, ko, :]),
                                    start=(ko == 0), stop=(ko == KO - 1),
                                )
                            nc.vector.tensor_scalar_add(
                                KT[:, p, n * 512:(n + 1) * 512], ps[:],
                                bk_sb[:, p:p + 1],
                            )
                        for j in range(4):
                            ps = ps12.tile([128, 512], F32, tag="v")
                            for ko in range(KO):
                                nc.tensor.matmul(
                                    ps[:, :DH],
                                    _r(xb[:, ko, j * 128:(j + 1) * 128]),
                                    _r(wv_sb[:, ko, :]),
                                    start=(ko == 0), stop=(ko == KO - 1),
                                )
                            nc.vector.tensor_copy(V[:, n * 4 + j, :], ps[:, :DH])

            # ---------------- phase 3: attention ----------------
            CHUNKS = [3] * 10 + [2]   # 32 key tiles in exp-sized chunks
            with tc.tile_pool(name="pt", bufs=3) as pt_pool, \
                 tc.tile_pool(name="accp", bufs=4) as acc_pool, \
                 tc.tile_pool(name="den", bufs=4) as den_pool, \
                 tc.tile_pool(name="ps3", bufs=1, space="PSUM") as ps3, \
                 tc.tile_pool(name="pso", bufs=2, space="PSUM") as pso:
                for p in range(NP):
                    for qh in range(SQ // 512):
                        qs = slice(qh * 512, (qh + 1) * 512)
                        o_ps = pso.tile([128, 512], F32, tag="out")
                        accA = acc_pool.tile([128, 512], F32, tag="accA")
                        accB = acc_pool.tile([128, 512], F32, tag="accB")
                        nc.vector.memset(accA[:], 0.0)
                        nc.vector.memset(accB[:], 0.0)
                        j0 = 0
                        for ci, cs in enumerate(CHUNKS):
                            scA = ps3.tile([128, 3, 512], F32, tag="scA")
                            scB = ps3.tile([128, 3, 512], F32, tag="scB")
                            for t in range(cs):
                                j = j0 + t
                                js = slice(j * 128, (j + 1) * 128)
                                nc.tensor.matmul(
                                    scA[:, t, :],
                                    _r(KT[0:64, p, js]), _r(QT[0:64, p, qs]),
                                    start=True, stop=True,
                                    tile_position=(0, 0),
                                )
                                nc.tensor.matmul(
                                    scB[:, t, :],
                                    _r(KT[64:128, p, js]), _r(QT[64:128, p, qs]),
                                    start=True, stop=True,
                                    tile_position=(64, 0),
                                )
                            ptA = pt_pool.tile([128, 3, 512], F32, tag="ptA")
                            ptB = pt_pool.tile([128, 3, 512], F32, tag="ptB")
                            nc.scalar.activation(
                                ptA[:, :cs, :], scA[:, :cs, :], AF.Exp, scale=SCALE
                            )
                            nc.scalar.activation(
                                ptB[:, :cs, :], scB[:, :cs, :], AF.Exp, scale=SCALE
                            )
                            for t in range(cs):
                                j = j0 + t
                                nc.tensor.matmul(
                                    o_ps[0:64, :],
                                    _r(V[:, j, p * 128:p * 128 + 64]),
                                    _r(ptA[:, t, :]),
                                    start=(j == 0), stop=(j == NJ - 1),
                                    tile_position=(0, 0),
                                    skip_group_check=True,
                                )
                                nc.tensor.matmul(
                                    o_ps[64:128, :],
                                    _r(V[:, j, p * 128 + 64:(p + 1) * 128]),
                                    _r(ptB[:, t, :]),
                                    start=(j == 0), stop=(j == NJ - 1),
                                    tile_position=(0, 64),
                                    skip_group_check=True,
                                )
                                nc.vector.tensor_add(accA[:], accA[:], ptA[:, t, :])
                                nc.vector.tensor_add(accB[:], accB[:], ptB[:, t, :])
                            j0 += cs
                        # denominators: broadcast-sum acc across partitions
                        dA_ps = pso.tile([128, 512], F32, tag="out")
                        nc.tensor.matmul(dA_ps[:], _r(ones_sb[:]), _r(accA[:]),
                                         start=True, stop=True)
                        dB_ps = pso.tile([128, 512], F32, tag="out")
                        nc.tensor.matmul(dB_ps[:], _r(ones_sb[:]), _r(accB[:]),
                                         start=True, stop=True)
                        dA = den_pool.tile([128, 512], F32, tag="dA")
                        dB = den_pool.tile([128, 512], F32, tag="dB")
                        nc.vector.reciprocal(dA[0:64, :], dA_ps[0:64, :])
                        nc.vector.reciprocal(dB[64:128, :], dB_ps[64:128, :])
                        nc.vector.tensor_mul(
                            yT[0:64, p, qs], o_ps[0:64, :], dA[0:64, :]
                        )
                        nc.vector.tensor_mul(
                            yT[64:128, p, qs], o_ps[64:128, :], dB[64:128, :]
                        )

                # ---------------- phase 4: output projection ----------------
                with tc.tile_pool(name="ph4", bufs=3) as ph4:
                    for m in range(SQ // 128):
                        ms = slice(m * 128, (m + 1) * 128)
                        ob = ph4.tile([128, D], F32, tag="ob")
                        for n, (n0, nw) in enumerate([(0, 512), (512, 256)]):
                            ps = pso.tile([128, 512], F32, tag="out")
                            for p in range(NP):
                                nc.tensor.matmul(
                                    ps[:, :nw],
                                    _r(yT[:, p, ms]),
                                    _r(ww_sb[:, p, n0:n0 + nw]),
                                    start=(p == 0), stop=(p == NP - 1),
                                )
                            nc.vector.tensor_copy(ob[:, n0:n0 + nw], ps[:, :nw])
                        nc.sync.dma_start(out[ms, :], ob[:])

    return nc


_NC_CACHE = None


def kernel(x, wq, bq, wk, bk, wv, bv, ww, bw):
    global _NC_CACHE
    if _NC_CACHE is None:
        _NC_CACHE = build_nc()
    nc = _NC_CACHE

    x = np.ascontiguousarray(np.asarray(x, dtype=np.float32))
    B, S_, D_ = x.shape
    assert (B, S_, D_) == (1, S, D)
    xT_full = np.ascontiguousarray(x[0].T)  # [D, S]

    in_maps = []
    for core in range(8):
        g, c = core // NC, core % NC
        gs = slice(g * DH, (g + 1) * DH)
        in_maps.append({
            "xT": xT_full,
            "xqT": np.ascontiguousarray(xT_full[:, c * SQ:(c + 1) * SQ]),
            "wqT": np.ascontiguousarray(wq[gs, :].T),
            "wkT": np.ascontiguousarray(wk[gs, :].T),
            "wvT": np.ascontiguousarray(wv[gs, :].T),
            "wwT": np.ascontiguousarray(ww[:, gs].T),
            "bq": np.ascontiguousarray(bq[gs].reshape(NP, 128).T),
            "bk": np.ascontiguousarray(bk[gs].reshape(NP, 128).T),
        })

    res = run_bass_kernel_spmd(nc, in_maps, core_ids=list(range(8)))

    const_row = (bv @ ww.T + bw).astype(np.float32)  # [768]
    out = np.empty((1, S, D), dtype=np.float32)
    for c in range(NC):
        acc = res.results[0 * NC + c]["out"] + res.results[1 * NC + c]["out"]
        out[0, c * SQ:(c + 1) * SQ, :] = acc + const_row
    return out


# revision 16
# speedup vs baseline: 1.0188x; 1.0188x over previous
"""Multi-head attention (B=1, S=4096, D=768, H=12, Hd=64) on 8 trn2 cores.

Sharding: 2 head-groups (6 heads = 384 dims, Megatron column-split wq/wk/wv,
row-split ww) x 4 query-chunks (1024 rows).  core = g*4 + c.
Each core returns a partial output [1024, 768]; host sums the 2 group
partials per chunk and adds (bv @ ww.T + bw).

Per-core plan:
  xT  [768, 4096]   x transposed (keys/values source), xqT = query columns.
  QT/KT [128, 3, *] head-pair-packed transposed projections: partition
                    l, pair p -> local dim p*128+l.  The two heads of a pair
                    run their scores matmuls concurrently in the PE array via
                    contraction row-packing (base partitions 0 / 64).
  V2  [128, 32, 6, 65]  value rows (key j on partitions) per head, with a
                    ones column at index 64: the attnV matmul (M=65) then
                    accumulates both out^T (rows 0-63) and the softmax
                    denominator (row 64) over key tiles in PSUM.
  scoresT psum [128 keys, 512 q]; exp on ACT engine psum->SBUF (x1/8 folded
                    into the activation scale; no max subtraction needed:
                    |scores| < 3).
  y6  [64, 6, 1024] normalized attn output^T per head (64 partitions), so
                    no partition shifts are needed; out-proj contracts 6x64.
All matmul inputs are float32r (full-rate fp32 mode, moving dim >= 256).
"""

import sys

if "/opt/trn_rl_repo" not in sys.path:
    sys.path.insert(0, "/opt/trn_rl_repo")

import numpy as np

import concourse.bacc as bacc
import concourse.bass as bass
import concourse.mybir as mybir
import concourse.tile as tile
from concourse.bass_utils import run_bass_kernel_spmd
from concourse.vector_clock import ScopedClock

F32 = mybir.dt.float32
F32R = mybir.dt.float32r

S = 4096          # sequence length
D = 768           # model dim
NG = 2            # head groups (cores axis 1)
NC = 4            # query chunks (cores axis 2)
DH = D // NG      # dims per group = 384
NP = DH // 128    # head pairs per group = 3
NH = 2 * NP       # heads per group = 6
SQ = S // NC      # queries per core = 1024
KO = D // 128     # contraction subtiles = 6
NJ = S // 128     # key tiles = 32
AF = mybir.ActivationFunctionType
SCALE = 0.125     # 1/sqrt(64)
CHUNKS = [3] * 10 + [2]   # 32 key tiles in exp-sized chunks

_PATCHED = False


def _patch_drain():
    """walrus in this container rejects >1 sync-wait per instruction
    ("Too many sync wait commands").  TileContext's tail drain aggregates one
    wait per live tile semaphore; redistribute them one-per-nop.  (Bacc's
    generate_event_semaphores handles the rest of the kernel.)"""
    global _PATCHED
    if _PATCHED:
        return
    _PATCHED = True

    def _drain_and_barrier(self, tick_clock, wait_clock):
        nc = self.nc
        drain_inst = nc.sync.drain()
        wait_clock.add_sem_waits(
            drain_inst.ins, ScopedClock({None: tick_clock.global_clock})
        )
        si = drain_inst.ins.sync_info
        waits = list(si.on_wait) if si is not None else []
        if len(waits) > 1:
            drain_inst.ins.sync_info = mybir.SyncInfo(
                on_wait=[waits[0]], on_update=list(si.on_update)
            )
            for w in waits[1:]:
                nop = nc.sync.nop(nofuse=True)
                nop.ins.sync_info = mybir.SyncInfo(on_wait=[w], on_update=[])
        nc.all_engine_barrier()
        assert self.sems is not None
        popped = nc._tile_sem_poison_stack.pop()
        assert popped is self._sem_poison
        nc.clear_and_free_semaphores(list(self.sems.allocated().values()))
        nc.all_engine_barrier()

    tile.TileContext._drain_and_barrier = _drain_and_barrier


def build_nc(loop_n=None):
    _patch_drain()
    nc = bacc.Bacc("TRN2", target_bir_lowering=False)

    xT = nc.dram_tensor("xT", [D, S], F32R, kind="ExternalInput")
    xqT = nc.dram_tensor("xqT", [D, SQ], F32R, kind="ExternalInput")
    wqT = nc.dram_tensor("wqT", [D, DH], F32R, kind="ExternalInput")
    wkT = nc.dram_tensor("wkT", [D, DH], F32R, kind="ExternalInput")
    wvT = nc.dram_tensor("wvT", [D, DH], F32R, kind="ExternalInput")
    wwT = nc.dram_tensor("wwT", [DH, D], F32R, kind="ExternalInput")
    bq = nc.dram_tensor("bq", [128, NP], F32, kind="ExternalInput")
    bk = nc.dram_tensor("bk", [128, NP], F32, kind="ExternalInput")
    out = nc.dram_tensor("out", [SQ, D], F32, kind="ExternalOutput")

    xT_r = xT.rearrange("(ko p) n -> p ko n", p=128)
    xqT_r = xqT.rearrange("(ko p) n -> p ko n", p=128)
    wqT_r = wqT.rearrange("(ko p) m -> p ko m", p=128)
    wkT_r = wkT.rearrange("(ko p) m -> p ko m", p=128)
    wvT_r = wvT.rearrange("(ko p) m -> p ko m", p=128)
    ww6_r = wwT.rearrange("(h l) o -> l h o", l=64)   # [64, 6, 768]

    with tile.TileContext(nc) as tc:
        import contextlib

        with contextlib.ExitStack() as ctx:
            if loop_n is not None:
                ctx.enter_context(tc.For_i(0, loop_n, 1))
            persist = ctx.enter_context(tc.tile_pool(name="persist", bufs=1))
            KT = persist.tile([128, NP, S], F32R)        # 48KB/part
            V2 = persist.tile([128, NJ, NH, 65], F32R)   # 48.75KB/part
            QT = persist.tile([128, NP, SQ], F32R)       # 12KB/part
            ones_f32 = persist.tile([128, 1], F32)
            nc.vector.memset(ones_f32[:], 1.0)
            # ones column of V2 (col 64 of every [j, h] slice)
            for j in range(NJ):
                nc.vector.tensor_copy(
                    V2[:, j, :, 64:65],
                    ones_f32[:, 0:1].to_broadcast((128, NH, 1)),
                )

            with tc.tile_pool(name="proj", bufs=1) as proj, \
                 tc.tile_pool(name="ps12", bufs=4, space="PSUM") as ps12:
                wk_sb = proj.tile([128, KO, DH], F32R)
                wv_sb = proj.tile([128, KO, DH], F32R)
                wq_sb = proj.tile([128, KO, DH], F32R)
                xq_sb = proj.tile([128, KO, SQ], F32R)
                bq_sb = proj.tile([128, NP], F32)
                bk_sb = proj.tile([128, NP], F32)
                nc.sync.dma_start(wk_sb[:], wkT_r[:])
                nc.sync.dma_start(bk_sb[:], bk[:])
                nc.sync.dma_start(wv_sb[:], wvT_r[:])

                # ------------- phase 1: K/V projections (stream xT) ------
                with tc.tile_pool(name="xstream", bufs=3) as xs_pool:
                    for n in range(S // 512):
                        xb = xs_pool.tile([128, KO, 512], F32R, tag="xb")
                        nc.sync.dma_start(xb[:], xT_r[:, :, n * 512:(n + 1) * 512])
                        for p in range(NP):
                            ps = ps12.tile([128, 512], F32, tag="qk")
                            for ko in range(KO):
                                nc.tensor.matmul(
                                    ps[:],
                                    wk_sb[:, ko, p * 128:(p + 1) * 128],
                                    xb[:, ko, :],
                                    start=(ko == 0), stop=(ko == KO - 1),
                                )
                            nc.vector.tensor_scalar_add(
                                KT[:, p, n * 512:(n + 1) * 512], ps[:],
                                bk_sb[:, p:p + 1],
                            )
                        for j4 in range(4):
                            j = n * 4 + j4
                            ps = ps12.tile([128, 512], F32, tag="v")
                            for ko in range(KO):
                                nc.tensor.matmul(
                                    ps[:, :DH],
                                    xb[:, ko, j4 * 128:(j4 + 1) * 128],
                                    wv_sb[:, ko, :],
                                    start=(ko == 0), stop=(ko == KO - 1),
                                )
                            for h in range(NH):
                                nc.vector.tensor_copy(
                                    V2[:, j, h, 0:64],
                                    ps[:, h * 64:(h + 1) * 64],
                                )
                        if n == 0:
                            # deferred so they don't delay the first x block
                            nc.sync.dma_start(wq_sb[:], wqT_r[:])
                            nc.sync.dma_start(xq_sb[:], xqT_r[:])
                            nc.sync.dma_start(bq_sb[:], bq[:])

                # ---------------- phase 2: Q projection -> QT ------------
                for p in range(NP):
                    for n in range(SQ // 512):
                        ps = ps12.tile([128, 512], F32, tag="qk")
                        for ko in range(KO):
                            nc.tensor.matmul(
                                ps[:],
                                wq_sb[:, ko, p * 128:(p + 1) * 128],
                                xq_sb[:, ko, n * 512:(n + 1) * 512],
                                start=(ko == 0), stop=(ko == KO - 1),
                            )
                        nc.vector.tensor_scalar_add(
                            QT[:, p, n * 512:(n + 1) * 512], ps[:],
                            bq_sb[:, p:p + 1],
                        )

            # ---------------- phases 3+4 ----------------
            with tc.tile_pool(name="late", bufs=1) as late, \
                 tc.tile_pool(name="pt", bufs=2) as pt_pool, \
                 tc.tile_pool(name="dn", bufs=2) as dn_pool, \
                 tc.tile_pool(name="bc", bufs=2) as bc_pool, \
                 tc.tile_pool(name="ob", bufs=2) as ob_pool, \
                 tc.tile_pool(name="ps_sc", bufs=1, space="PSUM") as ps_sc, \
                 tc.tile_pool(name="ps_out", bufs=1, space="PSUM") as ps_out:
                y6 = late.tile([64, NH, SQ], F32R)       # 24KB/part
                ww6 = late.tile([64, NH, D], F32R)
                nc.sync.dma_start(ww6[:], ww6_r[:])

                for qh in range(SQ // 512):
                    for p in range(NP):
                        qs = slice(qh * 512, (qh + 1) * 512)
                        oA = ps_out.tile([128, 512], F32, tag="outA")
                        oB = ps_out.tile([128, 512], F32, tag="outB")
                        j0 = 0
                        for cs in CHUNKS:
                            scA = ps_sc.tile([128, 3, 512], F32, tag="scA")
                            scB = ps_sc.tile([128, 3, 512], F32, tag="scB")
                            for t in range(cs):
                                j = j0 + t
                                js = slice(j * 128, (j + 1) * 128)
                                nc.tensor.matmul(
                                    scA[:, t, :],
                                    KT[0:64, p, js], QT[0:64, p, qs],
                                    start=True, stop=True,
                                    tile_position=(0, 0),
                                )
                                nc.tensor.matmul(
                                    scB[:, t, :],
                                    KT[64:128, p, js], QT[64:128, p, qs],
                                    start=True, stop=True,
                                    tile_position=(64, 0),
                                )
                            ptA = pt_pool.tile([128, 3, 512], F32R, tag="ptA")
                            ptB = pt_pool.tile([128, 3, 512], F32R, tag="ptB")
                            nc.scalar.activation(
                                ptA[:, :cs, :], scA[:, :cs, :], AF.Exp, scale=SCALE
                            )
                            nc.scalar.activation(
                                ptB[:, :cs, :], scB[:, :cs, :], AF.Exp, scale=SCALE
                            )
                            for t in range(cs):
                                j = j0 + t
                                nc.tensor.matmul(
                                    oA[0:65, :],
                                    V2[:, j, 2 * p, :], ptA[:, t, :],
                                    start=(j == 0), stop=(j == NJ - 1),
                                )
                                nc.tensor.matmul(
                                    oB[0:65, :],
                                    V2[:, j, 2 * p + 1, :], ptB[:, t, :],
                                    start=(j == 0), stop=(j == NJ - 1),
                                )
                            j0 += cs
                        # normalize: row 64 holds the softmax denominator
                        for h, o_ps in ((2 * p, oA), (2 * p + 1, oB)):
                            dn = dn_pool.tile([1, 512], F32, tag="dn")
                            nc.vector.tensor_copy(dn[:], o_ps[64:65, :])
                            bc = bc_pool.tile([64, 512], F32, tag="bc")
                            nc.gpsimd.partition_broadcast(bc[:], dn[:], channels=64)
                            nc.vector.reciprocal(bc[:], bc[:])
                            nc.vector.tensor_mul(
                                y6[0:64, h, qs], o_ps[0:64, :], bc[:]
                            )

                    # ---------- phase 4: out-projection for this q-half ----
                    for m in range(qh * 4, (qh + 1) * 4):
                        ms = slice(m * 128, (m + 1) * 128)
                        ob = ob_pool.tile([128, D], F32, tag="ob")
                        for n0, nw in ((0, 512), (512, 256)):
                            ps = ps_out.tile([128, 512], F32, tag="outA")
                            for h in range(NH):
                                nc.tensor.matmul(
                                    ps[:, :nw],
                                    y6[:, h, ms],
                                    ww6[:, h, n0:n0 + nw],
                                    start=(h == 0), stop=(h == NH - 1),
                                )
                            nc.vector.tensor_copy(ob[:, n0:n0 + nw], ps[:, :nw])
                        nc.sync.dma_start(out[ms, :], ob[:])

    nc.finalize()  # Bacc.compile(): reg alloc + split multi-sem-waits
    return nc


_NC_CACHE = None


def make_in_maps(x, wq, bq, wk, bk, wv, ww):
    x = np.ascontiguousarray(np.asarray(x, dtype=np.float32))
    xT_full = np.ascontiguousarray(x[0].T)  # [D, S]
    in_maps = []
    for core in range(8):
        g, c = core // NC, core % NC
        gs = slice(g * DH, (g + 1) * DH)
        in_maps.append({
            "xT": xT_full,
            "xqT": np.ascontiguousarray(xT_full[:, c * SQ:(c + 1) * SQ]),
            "wqT": np.ascontiguousarray(wq[gs, :].T),
            "wkT": np.ascontiguousarray(wk[gs, :].T),
            "wvT": np.ascontiguousarray(wv[gs, :].T),
            "wwT": np.ascontiguousarray(ww[:, gs].T),
            "bq": np.ascontiguousarray(bq[gs].reshape(NP, 128).T),
            "bk": np.ascontiguousarray(bk[gs].reshape(NP, 128).T),
        })
    return in_maps


def kernel(x, wq, bq, wk, bk, wv, bv, ww, bw):
    global _NC_CACHE
    if _NC_CACHE is None:
        _NC_CACHE = build_nc()
    nc = _NC_CACHE

    in_maps = make_in_maps(x, wq, bq, wk, bk, wv, ww)
    res = run_bass_kernel_spmd(nc, in_maps, core_ids=list(range(8)))

    const_row = (bv @ ww.T + bw).astype(np.float32)  # [768]
    out = np.empty((1, S, D), dtype=np.float32)
    for c in range(NC):
        acc = res.results[0 * NC + c]["out"] + res.results[1 * NC + c]["out"]
        out[0, c * SQ:(c + 1) * SQ, :] = acc + const_row
    return out
